# revision 49
# baseline (speedup 1.0000x reference)
"""Trainium2 Bass kernel for nn_GroupConvolutionLayer2d.

Computation (see reference):
  xn = (x - mean(x, -1)) / (std(x, -1) + 1e-7)          # per-row normalize
  lm = circular_conv(lm_raw, gauss_filt(sigma=0.1))      # along last axis
  y[b, i, j] = sum_n lm[i, j, n] * xn[b, n]              # [16384, 32, 32]

Strategy: data-parallel over batch across 8 NeuronCores (2048 rows each).

v2 design notes (vs the PE-transpose baseline):
  * Normalization is linear, so it is applied AFTER the matmul:
      y[b,p] = inv_b * (z[b,p] - mu_b * s[p]),
      z = x @ lmT,  s[p] = sum_n lmT[n,p] (= row-sums of lm_raw, since the
      Gaussian filter sums to 1).
    This lets the host pre-transpose x (layout only) so the main-matmul
    stationary tiles come straight from DMA: no PE transposes, no PSUM
    round-trip for xnT, and nothing cross-engine on the PE critical path.
  * The 33-tap circular conv is a banded-circulant matmul. Rolling lm_rawT
    by +16 rows (host, layout only) aligns the band so each 128-chunk of
    output needs only TWO stationary blocks (main band + wrap corner)
    instead of three.
  * s is computed on device: per-chunk adds on the vector engine, then a
    ones-column matmul + rank-1 broadcast on the PE (~0.9us).
  * inv_b is fused into the PSUM->SBUF copy (scalar engine activation
    scale); t1 = (mu*inv)*s is precomputed on the scalar engine during the
    matmuls (it does not depend on z); the subtract runs on the vector
    engine.  gpsimd only does tiny [128,1] ops (bulk gpsimd elementwise is
    ~20x slower than DVE).
  * Inputs stream on the sync HW DMA queue (cb + lmroll chunks first so
    the conv can start early; xt[0] early so tile-0's matmuls interleave
    with the conv tail); y goes out in bf16, also on the sync queue, which
    is idle once inputs are issued.
  * The PE warms up on dummy matmuls until the conv's DMA deps land (PE
    idle resets the HAM 1.2->2.4 GHz boost), and trailing dummy matmuls
    keep the clock boosted through the post-tail.
"""

import os
import sys

import numpy as np

for _p in ("/opt/trn_rl_repo",):
    if _p not in sys.path and os.path.isdir(_p):
        sys.path.insert(0, _p)

import ml_dtypes  # noqa: E402

import concourse.bass as bass  # noqa: E402
import concourse.mybir as mybir  # noqa: E402
import concourse.tile as tile  # noqa: E402
from concourse import bacc  # noqa: E402
from concourse.bass_utils import run_bass_kernel_spmd  # noqa: E402

N_CORES = 8
B_FULL = 16384
BS = B_FULL // N_CORES  # 2048 rows per core
NIN = 1024
P = 1024  # 32*32 output grid, flattened
NT = BS // 128  # 16 b-tiles per core
KT = NIN // 128  # 8 contraction chunks
FILT = 33
PAD = FILT // 2  # 16
SIGMA0 = 0.1
EPS = 1e-7

BF16 = ml_dtypes.bfloat16


def _gauss_filt() -> np.ndarray:
    t = (np.arange(FILT, dtype=np.float32) - FILT // 2) * np.float32(2.0 / FILT)
    k = np.exp(-0.5 * np.square(t / np.float32(SIGMA0)))
    return (k / k.sum()).astype(np.float32)


def _cb_blocks() -> np.ndarray:
    """Stationary blocks for the rolled banded-circulant conv matmul.

    lm[p, n] = sum_t filt[t] * lm_raw[p, (n + t - 16) % 1024]
    With lmroll[m'] = lm_rawT[(m' - 16) % 1024] the weight linking rolled
    row m' to output n is filt[m' - n], m' - n in [0, 32].  For output
    chunk ni the contributing m' live in chunks ni (B0) and ni+1 (B1):
      B0[mh, nh] = filt[mh - nh]        for 0 <= mh - nh <= 32
      B1[mh, nh] = filt[mh + 128 - nh]  for 0 <= mh + 128 - nh <= 32
    """
    filt = _gauss_filt()
    mh = np.arange(128)[:, None]
    nh = np.arange(128)[None, :]
    out = np.zeros((2, 128, 128), dtype=np.float32)
    d0 = mh - nh
    out[0] = np.where((d0 >= 0) & (d0 < FILT), filt[np.clip(d0, 0, FILT - 1)], 0.0)
    d1 = mh + 128 - nh
    out[1] = np.where((d1 >= 0) & (d1 < FILT), filt[np.clip(d1, 0, FILT - 1)], 0.0)
    return out


_CB = _cb_blocks().astype(BF16)


def _build_kernel_body(tc: "tile.TileContext", y_ap, xt_ap, xrow_ap, lmroll_ap, cb_ap):
    nc = tc.nc
    f32 = mybir.dt.float32
    bf16 = mybir.dt.bfloat16

    with (
        tc.tile_pool(name="const", bufs=1) as const_pool,
        tc.tile_pool(name="lm", bufs=1) as lm_pool,
        tc.tile_pool(name="xin", bufs=1) as xin_pool,
        tc.tile_pool(name="stat", bufs=16) as stat_pool,
        tc.tile_pool(name="yout", bufs=6) as y_pool,
        tc.tile_pool(name="t1p", bufs=3) as t1_pool,
        tc.tile_pool(name="pz0p", bufs=1, space="PSUM") as pz0_pool,
        tc.tile_pool(name="pmm", bufs=3, space="PSUM") as pmm_pool,
    ):
        # ---- constants / big SBUF staging ----
        cb_sb = const_pool.tile([128, 2, 128], bf16)
        ident = const_pool.tile([128, 128], bf16)
        ones_col = const_pool.tile([128, 1], bf16)
        ones_row = const_pool.tile([1, 128], bf16)
        s_row = const_pool.tile([1, P], bf16)
        usum = const_pool.tile([128, P], bf16)
        s_bcast = const_pool.tile([128, P], bf16)

        lmroll_sb = lm_pool.tile([128, KT, P], bf16)
        lmT_sb = lm_pool.tile([128, KT, P], bf16)
        xt_sb = lm_pool.tile([128, NT, KT, 128], bf16)
        xrow_sb = xin_pool.tile([128, NT, NIN], bf16)

        # warmup "stationary" only needs initialized data, not a true
        # identity; the vector engine is free ~2.5us before gpsimd
        nc.vector.memset(ident, 1.0)
        nc.gpsimd.memset(ones_col, 1.0)
        nc.gpsimd.memset(ones_row, 1.0)
        # dummy Sqrt preloads the scalar engine's activation table during
        # the idle prologue (otherwise a 1.3us ACT_TABLE_LOAD lands mid-run
        # right before the first real sqrt)
        sqwarm = const_pool.tile([1, 1], f32)
        nc.gpsimd.memset(sqwarm, 1.0)
        nc.scalar.activation(
            out=sqwarm, in_=sqwarm, func=mybir.ActivationFunctionType.Sqrt
        )

        # ---- input DMA issue order (sync HW queue): conv stationaries first,
        # then lmroll chunks so the conv can start as early as possible, then
        # x tiles.
        for s in range(2):
            nc.sync.dma_start(out=cb_sb[:, s, :], in_=cb_ap[s])
        for mi in range(4):
            nc.sync.dma_start(
                out=lmroll_sb[:, mi, :], in_=lmroll_ap[mi * 128 : (mi + 1) * 128, :]
            )
        # xt[0] early: tile-0's matmuls interleave with the conv tail
        nc.sync.dma_start(out=xt_sb[:, 0], in_=xt_ap[:, 0])
        for mi in range(4, KT):
            nc.sync.dma_start(
                out=lmroll_sb[:, mi, :], in_=lmroll_ap[mi * 128 : (mi + 1) * 128, :]
            )
        # interleave xt (stationaries) and xrow (stats) tiles
        nc.sync.dma_start(out=xt_sb[:, 1], in_=xt_ap[:, 1])
        for t in range(NT):
            if t + 2 < NT:
                nc.sync.dma_start(out=xt_sb[:, t + 2], in_=xt_ap[:, t + 2])
            nc.sync.dma_start(
                out=xrow_sb[:, t, :], in_=xrow_ap[t * 128 : (t + 1) * 128, :]
            )

        def mm_tile0(pz0, ni):
            for h in range(2):
                sl = slice(h * 512, (h + 1) * 512)
                nc.tensor.matmul(
                    pz0[:, sl],
                    lhsT=xt_sb[:, 0, ni, :],
                    rhs=lmT_sb[:, ni, sl],
                    start=(ni == 0),
                    stop=(ni == KT - 1),
                )

        # ---- PE warm-up: dummy matmuls until the conv's DMA deps land, so
        # the HAM clock boost (1.2 -> 2.4 GHz) engages before real work and
        # the PE never idles in between (idle resets the boost).
        pw = pmm_pool.tile([128, P], f32, tag="mm")
        for _ in range(42):
            nc.tensor.matmul(
                pw[:, 0:128], lhsT=ident, rhs=ident, start=True, stop=True
            )

        # ---- partial column sums of lmroll on the vector engine (free in
        # the prologue window); reduced across partitions on the PE below.
        nc.vector.tensor_tensor(
            out=usum,
            in0=lmroll_sb[:, 0, :],
            in1=lmroll_sb[:, 1, :],
            op=mybir.AluOpType.add,
        )
        for mi in range(2, KT):
            nc.vector.tensor_tensor(
                out=usum, in0=usum, in1=lmroll_sb[:, mi, :], op=mybir.AluOpType.add
            )

        # ---- banded conv matmul: lmT[ni] = B0.T @ lmroll[ni] + B1.T @ lmroll[ni+1]
        # The tail of the conv interleaves tile-0's main matmuls (dedicated
        # PSUM pool so the pmm rotation can't deadlock) to fill the
        # s-reduction latency gaps.
        pz0 = pz0_pool.tile([128, P], f32, tag="pz0")

        def conv_chunk(ni):
            pc = pmm_pool.tile([128, P], f32, tag="mm")
            for h in range(2):
                sl = slice(h * 512, (h + 1) * 512)
                nc.tensor.matmul(
                    pc[:, sl],
                    lhsT=cb_sb[:, 0, :],
                    rhs=lmroll_sb[:, ni, sl],
                    start=True,
                    stop=False,
                )
            for h in range(2):
                sl = slice(h * 512, (h + 1) * 512)
                nc.tensor.matmul(
                    pc[:, sl],
                    lhsT=cb_sb[:, 1, :],
                    rhs=lmroll_sb[:, (ni + 1) % KT, sl],
                    start=False,
                    stop=True,
                )
            # lmT copies mostly on the scalar engine (idle during the conv
            # phase) so the vector engine's usum adds are never queued
            # behind them (the PE's s-matmul waits on usum).  The last two
            # go to vector -- by then the adds are done, and the scalar
            # stream has fallen ~1us behind the conv pace, gating the
            # pz-slot rotation for the first main tiles.
            if ni >= KT - 3:
                nc.vector.tensor_scalar_add(out=lmT_sb[:, ni, :], in0=pc, scalar1=0.0)
            else:
                nc.scalar.copy(out=lmT_sb[:, ni, :], in_=pc)

        for ni in range(4):
            conv_chunk(ni)
        for ni in range(4, KT):
            conv_chunk(ni)
            mm_tile0(pz0, ni - 4)

        # ---- s[p] = sum over partitions of usum, via ones-column matmul,
        # then a rank-1 broadcast back across partitions (tiny PE work).
        ps = pmm_pool.tile([128, P], f32, tag="mm")
        for h in range(2):
            sl = slice(h * 512, (h + 1) * 512)
            nc.tensor.matmul(
                ps[0:1, sl], lhsT=ones_col, rhs=usum[:, sl], start=True, stop=True
            )
        nc.scalar.copy(out=s_row, in_=ps[0:1, :])
        mm_tile0(pz0, 4)
        mm_tile0(pz0, 5)
        psb = pmm_pool.tile([128, P], f32, tag="mm")
        for h in range(2):
            sl = slice(h * 512, (h + 1) * 512)
            nc.tensor.matmul(
                psb[:, sl], lhsT=ones_row, rhs=s_row[:, sl], start=True, stop=True
            )
        nc.vector.tensor_scalar_add(out=s_bcast, in0=psb, scalar1=0.0)
        mm_tile0(pz0, 6)
        mm_tile0(pz0, 7)

        # ---- per-row stats (vector/scalar engines; overlap the conv/main) ----
        def emit_stats(t, invs, cs):
            st = stat_pool.tile([128, 2, 6], f32, tag="st")
            nc.vector.bn_stats(out=st[:, 0, :], in_=xrow_sb[:, t, 0:512])
            nc.vector.bn_stats(out=st[:, 1, :], in_=xrow_sb[:, t, 512:1024])
            mv = stat_pool.tile([128, 2], f32, tag="mv")
            nc.vector.bn_aggr(out=mv, in_=st)
            sd = stat_pool.tile([128, 1], f32, tag="sd")
            nc.scalar.activation(
                out=sd, in_=mv[:, 1:2], func=mybir.ActivationFunctionType.Sqrt
            )
            # EPS=1e-7 on sd~1.0 is far below bf16 noise; skip the add
            inv = stat_pool.tile([128, 1], f32, tag="inv")
            nc.vector.reciprocal(out=inv, in_=sd)
            c = stat_pool.tile([128, 1], f32, tag="c")
            nc.gpsimd.tensor_tensor(
                out=c, in0=mv[:, 0:1], in1=inv, op=mybir.AluOpType.mult
            )
            invs.append(inv)
            cs.append(c)

        invs = []
        cs = []
        emit_stats(0, invs, cs)
        emit_stats(1, invs, cs)

        # ---- main matmul: z_t = x_t @ lmT; y_t = inv*(z_t - mu*s) ----
        for t in range(NT):
            if t + 2 < NT:
                emit_stats(t + 2, invs, cs)
            # t1 = (mu*inv) * s does not depend on z: compute it during the
            # matmuls (scalar engine activation with per-partition scale)
            t1 = t1_pool.tile([128, P], bf16)
            nc.scalar.activation(
                out=t1,
                in_=s_bcast,
                func=mybir.ActivationFunctionType.Copy,
                scale=cs[t],
            )
            if t == 0:
                pz = pz0  # matmuls already interleaved into the conv phase
            else:
                pz = pmm_pool.tile([128, P], f32, tag="mm")
                for ni in range(KT):
                    for h in range(2):
                        sl = slice(h * 512, (h + 1) * 512)
                        nc.tensor.matmul(
                            pz[:, sl],
                            lhsT=xt_sb[:, t, ni, :],
                            rhs=lmT_sb[:, ni, sl],
                            start=(ni == 0),
                            stop=(ni == KT - 1),
                        )
            # y = z*inv (scalar, fused into the PSUM->SBUF copy) - t1 (vector)
            yo = y_pool.tile([128, P], bf16)
            if t == NT - 1:
                # last tile: halves in parallel across engines to shorten the
                # end-of-kernel chain
                nc.scalar.activation(
                    out=yo[:, 0:512],
                    in_=pz[:, 0:512],
                    func=mybir.ActivationFunctionType.Copy,
                    scale=invs[t],
                )
                nc.vector.tensor_scalar(
                    out=yo[:, 512:1024],
                    in0=pz[:, 512:1024],
                    scalar1=invs[t],
                    scalar2=None,
                    op0=mybir.AluOpType.mult,
                )
                nc.vector.tensor_tensor(
                    out=yo[:, 512:1024],
                    in0=yo[:, 512:1024],
                    in1=t1[:, 512:1024],
                    op=mybir.AluOpType.subtract,
                )
                # issue the h1 half-DMA immediately; h0 follows its sub
                nc.sync.dma_start(
                    out=y_ap[t * 128 : (t + 1) * 128, 512:1024],
                    in_=yo[:, 512:1024],
                )
                nc.vector.tensor_tensor(
                    out=yo[:, 0:512],
                    in0=yo[:, 0:512],
                    in1=t1[:, 0:512],
                    op=mybir.AluOpType.subtract,
                )
                nc.sync.dma_start(
                    out=y_ap[t * 128 : (t + 1) * 128, 0:512], in_=yo[:, 0:512]
                )
            else:
                nc.scalar.activation(
                    out=yo,
                    in_=pz,
                    func=mybir.ActivationFunctionType.Copy,
                    scale=invs[t],
                )
                nc.vector.tensor_tensor(
                    out=yo, in0=yo, in1=t1, op=mybir.AluOpType.subtract
                )
            # y-out DMA issue on the sync queue (idle once inputs are issued)
            if t != NT - 1:
                nc.sync.dma_start(out=y_ap[t * 128 : (t + 1) * 128, :], in_=yo)

        # trailing dummy matmuls keep the PE loaded while the post-tail
        # copies/DMAs finish, so the HAM clock stays boosted into the drain.
        # pz0_pool has been free since tile 0 was copied out, so these don't
        # wait on the pmm rotation.
        pend = pz0_pool.tile([128, P], f32, tag="pz0")
        for _ in range(36):
            nc.tensor.matmul(
                pend[:, 0:128], lhsT=ident, rhs=ident, start=True, stop=True
            )


_NC_CACHE = None


def _get_nc():
    global _NC_CACHE
    if _NC_CACHE is None:
        nc = bacc.Bacc(
            "TRN2", target_bir_lowering=False, debug=False, num_devices=N_CORES
        )
        xt = nc.dram_tensor(
            "xt", [128, NT, KT, 128], mybir.dt.bfloat16, kind="ExternalInput"
        ).ap()
        xrow = nc.dram_tensor(
            "xrow", [BS, NIN], mybir.dt.bfloat16, kind="ExternalInput"
        ).ap()
        lmroll = nc.dram_tensor(
            "lmroll", [NIN, P], mybir.dt.bfloat16, kind="ExternalInput"
        ).ap()
        cb = nc.dram_tensor(
            "cb", [2, 128, 128], mybir.dt.bfloat16, kind="ExternalInput"
        ).ap()
        y = nc.dram_tensor("y", [BS, P], mybir.dt.bfloat16, kind="ExternalOutput").ap()
        with tile.TileContext(nc) as tc:
            _build_kernel_body(tc, y, xt, xrow, lmroll, cb)
        nc.compile()
        _NC_CACHE = nc
    return _NC_CACHE


def _in_maps(x: np.ndarray, lm_raw: np.ndarray):
    xb = np.asarray(x, dtype=np.float32).astype(BF16)  # [16384, 1024] bf16
    # per-core stationary layout: xt[nh, t, ni, bh] = x[c*2048 + t*128 + bh,
    # ni*128 + nh]  (pure layout transform + cast)
    lmr = np.ascontiguousarray(lm_raw, dtype=np.float32).reshape(P, NIN)
    lmroll = np.ascontiguousarray(np.roll(lmr.T, PAD, axis=0)).astype(BF16)
    maps = []
    for c in range(N_CORES):
        xs = xb[c * BS : (c + 1) * BS]  # [2048, 1024] bf16
        xtile = np.ascontiguousarray(
            xs.reshape(NT, 128, KT, 128).transpose(3, 0, 2, 1)
        )  # [128, 16, 8, 128]
        maps.append(
            {
                "xt": xtile,
                "xrow": np.ascontiguousarray(xs),
                "lmroll": lmroll,
                "cb": _CB,
            }
        )
    return maps


def run_spmd(x: np.ndarray, lm_raw: np.ndarray, **kwargs):
    """Run the device kernel; returns (y_full, BassKernelResults)."""
    res = run_bass_kernel_spmd(
        _get_nc(), _in_maps(x, lm_raw), core_ids=list(range(N_CORES)), **kwargs
    )
    y = np.concatenate([r["y"] for r in res.results], axis=0)
    return y.reshape(B_FULL, 32, 32).astype(np.float32), res


def kernel(x: np.ndarray, lm_raw: np.ndarray) -> np.ndarray:
    y, _ = run_spmd(x, lm_raw)
    return y


# revision 50
# speedup vs baseline: 1.0213x; 1.0213x over previous
"""Trainium2 Bass kernel for nn_GroupConvolutionLayer2d.

Computation (see reference):
  xn = (x - mean(x, -1)) / (std(x, -1) + 1e-7)          # per-row normalize
  lm = circular_conv(lm_raw, gauss_filt(sigma=0.1))      # along last axis
  y[b, i, j] = sum_n lm[i, j, n] * xn[b, n]              # [16384, 32, 32]

Strategy: data-parallel over batch across 8 NeuronCores (2048 rows each).

v2 design notes (vs the PE-transpose baseline):
  * Normalization is linear, so it is applied AFTER the matmul:
      y[b,p] = inv_b * (z[b,p] - mu_b * s[p]),
      z = x @ lmT,  s[p] = sum_n lmT[n,p] (= row-sums of lm_raw, since the
      Gaussian filter sums to 1).
    This lets the host pre-transpose x (layout only) so the main-matmul
    stationary tiles come straight from DMA: no PE transposes, no PSUM
    round-trip for xnT, and nothing cross-engine on the PE critical path.
  * The 33-tap circular conv is a banded-circulant matmul. Rolling lm_rawT
    by +16 rows (host, layout only) aligns the band so each 128-chunk of
    output needs only TWO stationary blocks (main band + wrap corner)
    instead of three.
  * s is computed on device: per-chunk adds on the vector engine, then a
    ones-column matmul + rank-1 broadcast on the PE (~0.9us).
  * inv_b is fused into the PSUM->SBUF copy (scalar engine activation
    scale); t1 = (mu*inv)*s is precomputed on the scalar engine during the
    matmuls (it does not depend on z); the subtract runs on the vector
    engine.  gpsimd only does tiny [128,1] ops (bulk gpsimd elementwise is
    ~20x slower than DVE).
  * Inputs stream on the sync HW DMA queue (cb + lmroll chunks first so
    the conv can start early; xt[0] early so tile-0's matmuls interleave
    with the conv tail); y goes out in bf16, also on the sync queue, which
    is idle once inputs are issued.
  * The PE warms up on dummy matmuls until the conv's DMA deps land (PE
    idle resets the HAM 1.2->2.4 GHz boost), and trailing dummy matmuls
    keep the clock boosted through the post-tail.
"""

import os
import sys

import numpy as np

for _p in ("/opt/trn_rl_repo",):
    if _p not in sys.path and os.path.isdir(_p):
        sys.path.insert(0, _p)

import ml_dtypes  # noqa: E402

import concourse.bass as bass  # noqa: E402
import concourse.mybir as mybir  # noqa: E402
import concourse.tile as tile  # noqa: E402
from concourse import bacc  # noqa: E402
from concourse.bass_utils import run_bass_kernel_spmd  # noqa: E402

N_CORES = 8
B_FULL = 16384
BS = B_FULL // N_CORES  # 2048 rows per core
NIN = 1024
P = 1024  # 32*32 output grid, flattened
NT = BS // 128  # 16 b-tiles per core
KT = NIN // 128  # 8 contraction chunks
FILT = 33
PAD = FILT // 2  # 16
SIGMA0 = 0.1
EPS = 1e-7

BF16 = ml_dtypes.bfloat16


def _gauss_filt() -> np.ndarray:
    t = (np.arange(FILT, dtype=np.float32) - FILT // 2) * np.float32(2.0 / FILT)
    k = np.exp(-0.5 * np.square(t / np.float32(SIGMA0)))
    return (k / k.sum()).astype(np.float32)


def _cb_blocks() -> np.ndarray:
    """Stationary blocks for the rolled banded-circulant conv matmul.

    lm[p, n] = sum_t filt[t] * lm_raw[p, (n + t - 16) % 1024]
    With lmroll[m'] = lm_rawT[(m' - 16) % 1024] the weight linking rolled
    row m' to output n is filt[m' - n], m' - n in [0, 32].  For output
    chunk ni the contributing m' live in chunks ni (B0) and ni+1 (B1):
      B0[mh, nh] = filt[mh - nh]        for 0 <= mh - nh <= 32
      B1[mh, nh] = filt[mh + 128 - nh]  for 0 <= mh + 128 - nh <= 32
    """
    filt = _gauss_filt()
    mh = np.arange(128)[:, None]
    nh = np.arange(128)[None, :]
    out = np.zeros((2, 128, 128), dtype=np.float32)
    d0 = mh - nh
    out[0] = np.where((d0 >= 0) & (d0 < FILT), filt[np.clip(d0, 0, FILT - 1)], 0.0)
    d1 = mh + 128 - nh
    out[1] = np.where((d1 >= 0) & (d1 < FILT), filt[np.clip(d1, 0, FILT - 1)], 0.0)
    return out


_CB = _cb_blocks().astype(BF16)


def _build_kernel_body(tc: "tile.TileContext", y_ap, xt_ap, xrow_ap, lmroll_ap, cb_ap):
    nc = tc.nc
    f32 = mybir.dt.float32
    bf16 = mybir.dt.bfloat16

    with (
        tc.tile_pool(name="const", bufs=1) as const_pool,
        tc.tile_pool(name="lm", bufs=1) as lm_pool,
        tc.tile_pool(name="xin", bufs=1) as xin_pool,
        tc.tile_pool(name="stat", bufs=16) as stat_pool,
        tc.tile_pool(name="yout", bufs=6) as y_pool,
        tc.tile_pool(name="t1p", bufs=3) as t1_pool,
        tc.tile_pool(name="pz0p", bufs=1, space="PSUM") as pz0_pool,
        tc.tile_pool(name="pmm", bufs=3, space="PSUM") as pmm_pool,
    ):
        # ---- constants / big SBUF staging ----
        cb_sb = const_pool.tile([128, 2, 128], bf16)
        ident = const_pool.tile([128, 128], bf16)
        ones_col = const_pool.tile([128, 1], bf16)
        ones_row = const_pool.tile([1, 128], bf16)
        s_row = const_pool.tile([1, P], bf16)
        usum = const_pool.tile([128, P], bf16)
        s_bcast = const_pool.tile([128, P], bf16)

        lmroll_sb = lm_pool.tile([128, KT, P], bf16)
        lmT_sb = lm_pool.tile([128, KT, P], bf16)
        xt_sb = lm_pool.tile([128, NT, KT, 128], bf16)
        xrow_sb = xin_pool.tile([128, NT, NIN], bf16)

        # warmup "stationary" only needs initialized data, not a true
        # identity; the vector engine is free ~2.5us before gpsimd
        nc.vector.memset(ident, 1.0)
        nc.gpsimd.memset(ones_col, 1.0)
        nc.gpsimd.memset(ones_row, 1.0)
        # dummy Sqrt preloads the scalar engine's activation table during
        # the idle prologue (otherwise a 1.3us ACT_TABLE_LOAD lands mid-run
        # right before the first real sqrt)
        sqwarm = const_pool.tile([1, 1], f32)
        nc.gpsimd.memset(sqwarm, 1.0)
        nc.scalar.activation(
            out=sqwarm, in_=sqwarm, func=mybir.ActivationFunctionType.Sqrt
        )

        # ---- input DMA issue order (sync HW queue): conv stationaries first,
        # then lmroll chunks so the conv can start as early as possible, then
        # x tiles.
        for s in range(2):
            nc.sync.dma_start(out=cb_sb[:, s, :], in_=cb_ap[s])
        for mi in range(4):
            nc.sync.dma_start(
                out=lmroll_sb[:, mi, :], in_=lmroll_ap[mi * 128 : (mi + 1) * 128, :]
            )
        # xt[0] early: tile-0's matmuls interleave with the conv tail
        nc.sync.dma_start(out=xt_sb[:, 0], in_=xt_ap[:, 0])
        for mi in range(4, KT):
            nc.sync.dma_start(
                out=lmroll_sb[:, mi, :], in_=lmroll_ap[mi * 128 : (mi + 1) * 128, :]
            )
        # interleave xt (stationaries) and xrow (stats) tiles
        nc.sync.dma_start(out=xt_sb[:, 1], in_=xt_ap[:, 1])
        for t in range(NT):
            if t + 2 < NT:
                nc.sync.dma_start(out=xt_sb[:, t + 2], in_=xt_ap[:, t + 2])
            nc.sync.dma_start(
                out=xrow_sb[:, t, :], in_=xrow_ap[t * 128 : (t + 1) * 128, :]
            )

        def mm_tile0(pz0, ni):
            for h in range(2):
                sl = slice(h * 512, (h + 1) * 512)
                nc.tensor.matmul(
                    pz0[:, sl],
                    lhsT=xt_sb[:, 0, ni, :],
                    rhs=lmT_sb[:, ni, sl],
                    start=(ni == 0),
                    stop=(ni == KT - 1),
                )

        # ---- PE warm-up: dummy matmuls until the conv's DMA deps land, so
        # the HAM clock boost (1.2 -> 2.4 GHz) engages before real work and
        # the PE never idles in between (idle resets the boost).
        pw = pmm_pool.tile([128, P], f32, tag="mm")
        for _ in range(42):
            nc.tensor.matmul(
                pw[:, 0:128], lhsT=ident, rhs=ident, start=True, stop=True
            )

        # ---- partial column sums of lmroll on the vector engine (free in
        # the prologue window); reduced across partitions on the PE below.
        nc.vector.tensor_tensor(
            out=usum,
            in0=lmroll_sb[:, 0, :],
            in1=lmroll_sb[:, 1, :],
            op=mybir.AluOpType.add,
        )
        for mi in range(2, KT):
            nc.vector.tensor_tensor(
                out=usum, in0=usum, in1=lmroll_sb[:, mi, :], op=mybir.AluOpType.add
            )

        # ---- banded conv matmul: lmT[ni] = B0.T @ lmroll[ni] + B1.T @ lmroll[ni+1]
        # The tail of the conv interleaves tile-0's main matmuls (dedicated
        # PSUM pool so the pmm rotation can't deadlock) to fill the
        # s-reduction latency gaps.
        pz0 = pz0_pool.tile([128, P], f32, tag="pz0")

        def conv_chunk(ni):
            pc = pmm_pool.tile([128, P], f32, tag="mm")
            for h in range(2):
                sl = slice(h * 512, (h + 1) * 512)
                nc.tensor.matmul(
                    pc[:, sl],
                    lhsT=cb_sb[:, 0, :],
                    rhs=lmroll_sb[:, ni, sl],
                    start=True,
                    stop=False,
                )
            for h in range(2):
                sl = slice(h * 512, (h + 1) * 512)
                nc.tensor.matmul(
                    pc[:, sl],
                    lhsT=cb_sb[:, 1, :],
                    rhs=lmroll_sb[:, (ni + 1) % KT, sl],
                    start=False,
                    stop=True,
                )
            # lmT copies mostly on the scalar engine (idle during the conv
            # phase) so the vector engine's usum adds are never queued
            # behind them (the PE's s-matmul waits on usum).  The last two
            # go to vector -- by then the adds are done, and the scalar
            # stream has fallen ~1us behind the conv pace, gating the
            # pz-slot rotation for the first main tiles.
            if ni >= KT - 2:
                nc.vector.tensor_scalar_add(out=lmT_sb[:, ni, :], in0=pc, scalar1=0.0)
            else:
                nc.scalar.copy(out=lmT_sb[:, ni, :], in_=pc)

        for ni in range(4):
            conv_chunk(ni)
        for ni in range(4, KT):
            conv_chunk(ni)
            mm_tile0(pz0, ni - 4)

        # ---- s[p] = sum over partitions of usum, via ones-column matmul,
        # then a rank-1 broadcast back across partitions (tiny PE work).
        ps = pmm_pool.tile([128, P], f32, tag="mm")
        for h in range(2):
            sl = slice(h * 512, (h + 1) * 512)
            nc.tensor.matmul(
                ps[0:1, sl], lhsT=ones_col, rhs=usum[:, sl], start=True, stop=True
            )
        nc.scalar.copy(out=s_row, in_=ps[0:1, :])
        mm_tile0(pz0, 4)
        mm_tile0(pz0, 5)
        psb = pmm_pool.tile([128, P], f32, tag="mm")
        for h in range(2):
            sl = slice(h * 512, (h + 1) * 512)
            nc.tensor.matmul(
                psb[:, sl], lhsT=ones_row, rhs=s_row[:, sl], start=True, stop=True
            )
        nc.vector.tensor_scalar_add(out=s_bcast, in0=psb, scalar1=0.0)
        mm_tile0(pz0, 6)
        mm_tile0(pz0, 7)

        # ---- per-row stats (vector/scalar engines; overlap the conv/main) ----
        def emit_stats(t, invs, cs):
            st = stat_pool.tile([128, 2, 6], f32, tag="st")
            nc.vector.bn_stats(out=st[:, 0, :], in_=xrow_sb[:, t, 0:512])
            nc.vector.bn_stats(out=st[:, 1, :], in_=xrow_sb[:, t, 512:1024])
            mv = stat_pool.tile([128, 2], f32, tag="mv")
            nc.vector.bn_aggr(out=mv, in_=st)
            sd = stat_pool.tile([128, 1], f32, tag="sd")
            nc.scalar.activation(
                out=sd, in_=mv[:, 1:2], func=mybir.ActivationFunctionType.Sqrt
            )
            # EPS=1e-7 on sd~1.0 is far below bf16 noise; skip the add
            inv = stat_pool.tile([128, 1], f32, tag="inv")
            nc.vector.reciprocal(out=inv, in_=sd)
            c = stat_pool.tile([128, 1], f32, tag="c")
            nc.gpsimd.tensor_tensor(
                out=c, in0=mv[:, 0:1], in1=inv, op=mybir.AluOpType.mult
            )
            invs.append(inv)
            cs.append(c)

        invs = []
        cs = []
        emit_stats(0, invs, cs)
        emit_stats(1, invs, cs)

        # ---- main matmul: z_t = x_t @ lmT; y_t = inv*(z_t - mu*s) ----
        for t in range(NT):
            if t + 2 < NT:
                emit_stats(t + 2, invs, cs)
            # t1 = (mu*inv) * s does not depend on z: compute it during the
            # matmuls (scalar engine activation with per-partition scale)
            t1 = t1_pool.tile([128, P], bf16)
            nc.scalar.activation(
                out=t1,
                in_=s_bcast,
                func=mybir.ActivationFunctionType.Copy,
                scale=cs[t],
            )
            if t == 0:
                pz = pz0  # matmuls already interleaved into the conv phase
            else:
                pz = pmm_pool.tile([128, P], f32, tag="mm")
                for ni in range(KT):
                    for h in range(2):
                        sl = slice(h * 512, (h + 1) * 512)
                        nc.tensor.matmul(
                            pz[:, sl],
                            lhsT=xt_sb[:, t, ni, :],
                            rhs=lmT_sb[:, ni, sl],
                            start=(ni == 0),
                            stop=(ni == KT - 1),
                        )
            # y = z*inv (scalar, fused into the PSUM->SBUF copy) - t1 (vector)
            yo = y_pool.tile([128, P], bf16)
            if t == NT - 1:
                # last tile: halves in parallel across engines to shorten the
                # end-of-kernel chain
                nc.scalar.activation(
                    out=yo[:, 0:512],
                    in_=pz[:, 0:512],
                    func=mybir.ActivationFunctionType.Copy,
                    scale=invs[t],
                )
                nc.vector.tensor_scalar(
                    out=yo[:, 512:1024],
                    in0=pz[:, 512:1024],
                    scalar1=invs[t],
                    scalar2=None,
                    op0=mybir.AluOpType.mult,
                )
                nc.vector.tensor_tensor(
                    out=yo[:, 512:1024],
                    in0=yo[:, 512:1024],
                    in1=t1[:, 512:1024],
                    op=mybir.AluOpType.subtract,
                )
                # issue the h1 half-DMA immediately; h0 follows its sub
                nc.sync.dma_start(
                    out=y_ap[t * 128 : (t + 1) * 128, 512:1024],
                    in_=yo[:, 512:1024],
                )
                nc.vector.tensor_tensor(
                    out=yo[:, 0:512],
                    in0=yo[:, 0:512],
                    in1=t1[:, 0:512],
                    op=mybir.AluOpType.subtract,
                )
                nc.sync.dma_start(
                    out=y_ap[t * 128 : (t + 1) * 128, 0:512], in_=yo[:, 0:512]
                )
            else:
                nc.scalar.activation(
                    out=yo,
                    in_=pz,
                    func=mybir.ActivationFunctionType.Copy,
                    scale=invs[t],
                )
                nc.vector.tensor_tensor(
                    out=yo, in0=yo, in1=t1, op=mybir.AluOpType.subtract
                )
            # y-out DMA issue on the sync queue (idle once inputs are issued)
            if t != NT - 1:
                nc.sync.dma_start(out=y_ap[t * 128 : (t + 1) * 128, :], in_=yo)

        # trailing dummy matmuls keep the PE loaded while the post-tail
        # copies/DMAs finish, so the HAM clock stays boosted into the drain.
        # pz0_pool has been free since tile 0 was copied out, so these don't
        # wait on the pmm rotation.
        pend = pz0_pool.tile([128, P], f32, tag="pz0")
        for _ in range(36):
            nc.tensor.matmul(
                pend[:, 0:128], lhsT=ident, rhs=ident, start=True, stop=True
            )


_NC_CACHE = None


def _get_nc():
    global _NC_CACHE
    if _NC_CACHE is None:
        nc = bacc.Bacc(
            "TRN2", target_bir_lowering=False, debug=False, num_devices=N_CORES
        )
        xt = nc.dram_tensor(
            "xt", [128, NT, KT, 128], mybir.dt.bfloat16, kind="ExternalInput"
        ).ap()
        xrow = nc.dram_tensor(
            "xrow", [BS, NIN], mybir.dt.bfloat16, kind="ExternalInput"
        ).ap()
        lmroll = nc.dram_tensor(
            "lmroll", [NIN, P], mybir.dt.bfloat16, kind="ExternalInput"
        ).ap()
        cb = nc.dram_tensor(
            "cb", [2, 128, 128], mybir.dt.bfloat16, kind="ExternalInput"
        ).ap()
        y = nc.dram_tensor("y", [BS, P], mybir.dt.bfloat16, kind="ExternalOutput").ap()
        with tile.TileContext(nc) as tc:
            _build_kernel_body(tc, y, xt, xrow, lmroll, cb)
        nc.compile()
        _NC_CACHE = nc
    return _NC_CACHE


def _in_maps(x: np.ndarray, lm_raw: np.ndarray):
    xb = np.asarray(x, dtype=np.float32).astype(BF16)  # [16384, 1024] bf16
    # per-core stationary layout: xt[nh, t, ni, bh] = x[c*2048 + t*128 + bh,
    # ni*128 + nh]  (pure layout transform + cast)
    lmr = np.ascontiguousarray(lm_raw, dtype=np.float32).reshape(P, NIN)
    lmroll = np.ascontiguousarray(np.roll(lmr.T, PAD, axis=0)).astype(BF16)
    maps = []
    for c in range(N_CORES):
        xs = xb[c * BS : (c + 1) * BS]  # [2048, 1024] bf16
        xtile = np.ascontiguousarray(
            xs.reshape(NT, 128, KT, 128).transpose(3, 0, 2, 1)
        )  # [128, 16, 8, 128]
        maps.append(
            {
                "xt": xtile,
                "xrow": np.ascontiguousarray(xs),
                "lmroll": lmroll,
                "cb": _CB,
            }
        )
    return maps


def run_spmd(x: np.ndarray, lm_raw: np.ndarray, **kwargs):
    """Run the device kernel; returns (y_full, BassKernelResults)."""
    res = run_bass_kernel_spmd(
        _get_nc(), _in_maps(x, lm_raw), core_ids=list(range(N_CORES)), **kwargs
    )
    y = np.concatenate([r["y"] for r in res.results], axis=0)
    return y.reshape(B_FULL, 32, 32).astype(np.float32), res


def kernel(x: np.ndarray, lm_raw: np.ndarray) -> np.ndarray:
    y, _ = run_spmd(x, lm_raw)
    return y
